# revision 7
# baseline (speedup 1.0000x reference)
"""CrossViewTransformer kernel for 8 Trainium2 NeuronCores.

Problem: B=4, C=256, H=W=64 (N=4096), Cqk=32 cross-attention + residual.
  Q = Wq@src, K = Wk@tgt, V = Wv@tgt  (1x1 convs over channels)
  out = softmax(Q^T K) @ V^T + src     (no 1/sqrt(d) scale)

Sharding: 8 cores = 4 batches x 2 query-halves. Each core computes attention
for 2048 queries x 4096 keys of one batch. The V projection is replicated
across the 2 cores of a batch (cheap: 0.5 GFLOP) while the expensive parts
(scores, exp, attn@V) are fully split.

Per-core layout (all on-chip once loaded):
  tgt   [256, 4096] bf16      (2 ch-chunk tiles of [128, ...])
  srcq  [256, 2048] bf16      (query-half slice of src, for Q projection)
  src_res [256, 2048] fp32    (residual)
  K_sb  [32, 4096]  bf16      K projection
  Q_sb  [32, 2048]  bf16      Q projection
  VT_sb [128, 32, 256] bf16   V^T, m-major tiles (VT[m, c])

Attention loop per q-chunk (512 queries):
  for each m-group (4 m-tiles of 128 keys):
    scoresT[m,q] tiles via matmul(lhsT=K_sb tile, rhs=Q_sb chunk) -> PSUM
    one Exp activation over [128, 2048] PSUM -> expT bf16 SBUF
    attn@V: matmul(lhsT=VT tile c-half, rhs=expT tile) accumulating [c,q]
    denominator: matmul(lhsT=ones[128,1], rhs=expT tile) accumulating [1,q]
  r = 1/l, broadcast across partitions, out = av * r + src_res -> DRAM
"""

import os
import sys

sys.path.insert(0, "/opt/trn_rl_repo")

import numpy as np
import ml_dtypes

BF16 = ml_dtypes.bfloat16

B, C, H, W = 4, 256, 64, 64
N = H * W            # 4096 keys (and queries per batch)
CQK = 32
NCORES = 8
QSH = N // 2         # 2048 queries per core
QC = 512             # q-chunk width (one PSUM bank)
NQC = QSH // QC      # 4 q-chunks
MT = 128             # m-tile (keys per scoresT tile)
NMT = N // MT        # 32 m-tiles
MG = int(os.environ.get("KERNEL_MG", "2"))  # m-tiles per exp group (exp FD = MG*QC)
NG = NMT // MG       # 8 groups

_last_results = None  # BassKernelResults of the most recent run (for test.py)


def _build_bass():
    import concourse.bass as bass
    import concourse.tile as tile
    from concourse import bacc, mybir

    f32 = mybir.dt.float32
    bf16 = mybir.dt.bfloat16

    nc = bacc.Bacc("TRN2")

    # ---- DRAM I/O ----
    tgt_d = nc.dram_tensor("tgt", [C, N], bf16, kind="ExternalInput")
    srcq_d = nc.dram_tensor("srcq", [C, QSH], bf16, kind="ExternalInput")
    srcr_d = nc.dram_tensor("srcr", [C, QSH], f32, kind="ExternalInput")
    wqT_d = nc.dram_tensor("wqT", [C, CQK], bf16, kind="ExternalInput")
    wkT_d = nc.dram_tensor("wkT", [C, CQK], bf16, kind="ExternalInput")
    wvT_d = nc.dram_tensor("wvT", [C, C], bf16, kind="ExternalInput")
    bqk_d = nc.dram_tensor("bqk", [CQK, 2], f32, kind="ExternalInput")
    bv_d = nc.dram_tensor("bv", [1, C], bf16, kind="ExternalInput")
    out_d = nc.dram_tensor("out", [C, QSH], f32, kind="ExternalOutput")

    ones_col_d = nc.inline_tensor(np.ones((128, 1), dtype=BF16), name="ones_col")
    ones_row_d = nc.inline_tensor(np.ones((1, 128), dtype=BF16), name="ones_row")

    with tile.TileContext(nc) as tc:
        with (
            tc.tile_pool(name="const", bufs=1) as const,
            tc.tile_pool(name="data", bufs=1) as data,
        ):
            # ---- ACT table warmup: a dependency-free Exp so walrus's
            # inserted ACT_TABLE_LOAD lands on an instruction with <=1 wait.
            warm = const.tile([1, 8], f32, tag="warm")
            nc.vector.memset(warm, 0.0)
            nc.scalar.activation(
                out=warm, in_=warm, func=mybir.ActivationFunctionType.Exp
            )

            # ---- constants / weights ----
            wq0 = const.tile([128, CQK], bf16, tag="wq0")
            wq1 = const.tile([128, CQK], bf16, tag="wq1")
            wk0 = const.tile([128, CQK], bf16, tag="wk0")
            wk1 = const.tile([128, CQK], bf16, tag="wk1")
            wv0 = const.tile([128, C], bf16, tag="wv0")
            wv1 = const.tile([128, C], bf16, tag="wv1")
            bqk = const.tile([CQK, 2], f32, tag="bqk")
            bvr = const.tile([1, C], bf16, tag="bvr")
            ones_col = const.tile([128, 1], bf16, tag="ones_col")
            ones_row = const.tile([1, 128], bf16, tag="ones_row")
            nc.sync.dma_start(out=wq0, in_=wqT_d[0:128, :])
            nc.sync.dma_start(out=wq1, in_=wqT_d[128:256, :])
            nc.sync.dma_start(out=wk0, in_=wkT_d[0:128, :])
            nc.sync.dma_start(out=wk1, in_=wkT_d[128:256, :])
            nc.sync.dma_start(out=wv0, in_=wvT_d[0:128, :])
            nc.sync.dma_start(out=wv1, in_=wvT_d[128:256, :])
            nc.sync.dma_start(out=bqk, in_=bqk_d[:, :])
            nc.sync.dma_start(out=bvr, in_=bv_d[:, :])
            nc.sync.dma_start(out=ones_col, in_=ones_col_d[:, :])
            nc.sync.dma_start(out=ones_row, in_=ones_row_d[:, :])

            # ---- big data tiles ----
            # tgt as [128, 8, 512] per ch-chunk; chunked DMA so projections can
            # start as soon as their slice arrives.
            tgt0 = data.tile([128, 8, 512], bf16, tag="tgt0")
            tgt1 = data.tile([128, 8, 512], bf16, tag="tgt1")
            for j in range(8):
                sl = slice(j * 512, (j + 1) * 512)
                nc.sync.dma_start(out=tgt0[:, j, :], in_=tgt_d[0:128, sl])
                nc.sync.dma_start(out=tgt1[:, j, :], in_=tgt_d[128:256, sl])
            srcq0 = data.tile([128, NQC, QC], bf16, tag="srcq0")
            srcq1 = data.tile([128, NQC, QC], bf16, tag="srcq1")
            srcr0 = data.tile([128, NQC, QC], f32, tag="srcr0")
            srcr1 = data.tile([128, NQC, QC], f32, tag="srcr1")
            for j in range(NQC):
                sl = slice(j * QC, (j + 1) * QC)
                nc.sync.dma_start(out=srcq0[:, j, :], in_=srcq_d[0:128, sl])
                nc.sync.dma_start(out=srcq1[:, j, :], in_=srcq_d[128:256, sl])
                nc.sync.dma_start(out=srcr0[:, j, :], in_=srcr_d[0:128, sl])
                nc.sync.dma_start(out=srcr1[:, j, :], in_=srcr_d[128:256, sl])

            K_sb = data.tile([CQK, NMT, MT], bf16, tag="K_sb")
            Q_sb = data.tile([CQK, NQC, QC], bf16, tag="Q_sb")
            VT_sb = data.tile([128, NMT, C], bf16, tag="VT_sb")

            # ---- projections ----
            with (
                tc.tile_pool(name="pv", bufs=3, space="PSUM") as pv,
                tc.tile_pool(name="pk", bufs=2, space="PSUM") as pk,
            ):
                # V^T tiles: VT[m,c] = sum_ch tgt[ch,m] WvT[ch,c]  (+ bv)
                for mt in range(NMT):
                    ps = pv.tile([128, C], f32, tag="psv")
                    j, o = divmod(mt * MT, 512)
                    lhs0 = tgt0[:, j, o : o + MT]
                    lhs1 = tgt1[:, j, o : o + MT]
                    nc.tensor.matmul(ps, lhsT=lhs0, rhs=wv0, start=True, stop=False)
                    nc.tensor.matmul(ps, lhsT=lhs1, rhs=wv1, start=False, stop=False)
                    nc.tensor.matmul(ps, lhsT=ones_row, rhs=bvr, start=False, stop=True)
                    nc.vector.tensor_copy(out=VT_sb[:, mt, :], in_=ps)
                # K: K[cqk, m] = sum_ch WkT[ch,cqk] tgt[ch,m]  (+ bk)
                for j in range(8):
                    ps = pk.tile([CQK, 512], f32, tag="psk")
                    nc.tensor.matmul(ps, lhsT=wk0, rhs=tgt0[:, j, :], start=True, stop=False)
                    nc.tensor.matmul(ps, lhsT=wk1, rhs=tgt1[:, j, :], start=False, stop=True)
                    nc.vector.tensor_scalar_add(
                        K_sb[:, 4 * j : 4 * (j + 1), :].rearrange("p a b -> p (a b)"),
                        ps,
                        bqk[:, 1:2],
                    )
                # Q: Q[cqk, n] = sum_ch WqT[ch,cqk] srcq[ch,n]  (+ bq)
                for j in range(NQC):
                    ps = pk.tile([CQK, QC], f32, tag="psq")
                    nc.tensor.matmul(ps, lhsT=wq0, rhs=srcq0[:, j, :], start=True, stop=False)
                    nc.tensor.matmul(ps, lhsT=wq1, rhs=srcq1[:, j, :], start=False, stop=True)
                    nc.vector.tensor_scalar_add(Q_sb[:, j, :], ps, bqk[:, 0:1])

            # ---- attention ----
            with (
                tc.tile_pool(name="ps_s", bufs=1, space="PSUM") as ps_s,
                tc.tile_pool(name="ps_av", bufs=1, space="PSUM") as ps_av,
                tc.tile_pool(name="ps_l", bufs=1, space="PSUM") as ps_l,
                tc.tile_pool(name="att", bufs=3) as att,
                tc.tile_pool(name="outp", bufs=4) as outp,
            ):
                for qc in range(NQC):
                    av0 = ps_av.tile([128, QC], f32, tag="av0")
                    av1 = ps_av.tile([128, QC], f32, tag="av1")
                    lrow = ps_l.tile([1, QC], f32, tag="lrow")
                    rhs_q = Q_sb[:, qc, :]
                    for g in range(NG):
                        S = ps_s.tile([128, MG, QC], f32, tag="S")
                        for i in range(MG):
                            mt = g * MG + i
                            nc.tensor.matmul(
                                S[:, i, :],
                                lhsT=K_sb[:, mt, :],
                                rhs=rhs_q,
                                start=True,
                                stop=True,
                            )
                        expT = att.tile([128, MG, QC], bf16, tag="expT")
                        nc.scalar.activation(
                            out=expT.rearrange("p a b -> p (a b)"),
                            in_=S.rearrange("p a b -> p (a b)"),
                            func=mybir.ActivationFunctionType.Exp,
                        )
                        for i in range(MG):
                            mt = g * MG + i
                            first = mt == 0
                            last = mt == NMT - 1
                            nc.tensor.matmul(
                                av0,
                                lhsT=VT_sb[:, mt, 0:128],
                                rhs=expT[:, i, :],
                                start=first,
                                stop=last,
                            )
                            nc.tensor.matmul(
                                av1,
                                lhsT=VT_sb[:, mt, 128:256],
                                rhs=expT[:, i, :],
                                start=first,
                                stop=last,
                            )
                            nc.tensor.matmul(
                                lrow,
                                lhsT=ones_col,
                                rhs=expT[:, i, :],
                                start=first,
                                stop=last,
                            )
                    # softmax denominator -> reciprocal -> broadcast
                    l_sb = outp.tile([1, QC], f32, tag="l_sb")
                    r_sb = outp.tile([1, QC], f32, tag="r_sb")
                    r_rep = outp.tile([128, QC], f32, tag="r_rep")
                    nc.vector.tensor_copy(out=l_sb, in_=lrow)
                    nc.vector.reciprocal_approx_fast(out=r_sb, in_=l_sb)
                    nc.gpsimd.partition_broadcast(r_rep, r_sb)
                    for ci, (av, srcr) in enumerate(((av0, srcr0), (av1, srcr1))):
                        o = outp.tile([128, QC], f32, tag=f"o{ci}")
                        nc.vector.tensor_mul(o, av, r_rep)
                        nc.vector.tensor_add(o, o, srcr[:, qc, :])
                        nc.sync.dma_start(
                            out=out_d[128 * ci : 128 * (ci + 1), qc * QC : (qc + 1) * QC],
                            in_=o,
                        )
    nc.compile()
    return nc


_cached = None


def _get_bass():
    global _cached
    if _cached is None:
        _cached = _build_bass()
    return _cached


def kernel(src_feat, tgt_feat, Wq, bq, Wk, bk, Wv, bv):
    """Full inputs in, full output out. Shards internally across 8 cores."""
    global _last_results
    from concourse.bass_utils import run_bass_kernel_spmd

    src = np.asarray(src_feat, dtype=np.float32).reshape(B, C, N)
    tgt = np.asarray(tgt_feat, dtype=np.float32).reshape(B, C, N)
    wqT = np.ascontiguousarray(np.asarray(Wq, np.float32).T).astype(BF16)
    wkT = np.ascontiguousarray(np.asarray(Wk, np.float32).T).astype(BF16)
    wvT = np.ascontiguousarray(np.asarray(Wv, np.float32).T).astype(BF16)
    bqk = np.stack(
        [np.asarray(bq, np.float32), np.asarray(bk, np.float32)], axis=1
    )  # [32, 2]
    bvr = np.asarray(bv, np.float32).reshape(1, C).astype(BF16)

    tgt_bf = tgt.astype(BF16)

    in_maps = []
    for c in range(NCORES):
        b, h = divmod(c, 2)
        qsl = slice(h * QSH, (h + 1) * QSH)
        in_maps.append(
            {
                "tgt": np.ascontiguousarray(tgt_bf[b]),
                "srcq": np.ascontiguousarray(src[b, :, qsl]).astype(BF16),
                "srcr": np.ascontiguousarray(src[b, :, qsl]),
                "wqT": wqT,
                "wkT": wkT,
                "wvT": wvT,
                "bqk": np.ascontiguousarray(bqk),
                "bv": bvr,
            }
        )

    nc = _get_bass()
    res = run_bass_kernel_spmd(
        nc,
        in_maps,
        core_ids=list(range(NCORES)),
        trace=bool(int(os.environ.get("KERNEL_TRACE", "0"))),
    )
    _last_results = res

    out = np.empty((B, C, N), dtype=np.float32)
    for c in range(NCORES):
        b, h = divmod(c, 2)
        out[b, :, h * QSH : (h + 1) * QSH] = res.results[c]["out"]
    return out.reshape(B, C, H, W)


# revision 11
# speedup vs baseline: 1.0226x; 1.0226x over previous
"""CrossViewTransformer kernel for 8 Trainium2 NeuronCores.

Problem: B=4, C=256, H=W=64 (N=4096), Cqk=32 cross-attention + residual.
  Q = Wq@src, K = Wk@tgt, V = Wv@tgt  (1x1 convs over channels)
  out = softmax(Q^T K) @ V^T + src     (no 1/sqrt(d) scale)

Sharding: 8 cores = 4 batches x 2 query-halves. Each core computes attention
for 2048 queries x 4096 keys of one batch. The V projection is replicated
across the 2 cores of a batch (cheap: 0.5 GFLOP) while the expensive parts
(scores, exp, attn@V) are fully split.

Per-core pipeline (per 512-query chunk, m = key index):
  scoresT[m,q]: 4 row-packed matmuls (K=32 contraction at row groups
    0/32/64/96) -> 4 PSUM banks;  one Exp over the group -> expT bf16 SBUF
  attn@V: matmul(lhsT=VT tile c-half, rhs=expT tile) accumulating [c,q]
  denominator: col-packed ones-matmuls -> rows {0,32,64,96} of one PSUM bank,
    folded by a select-row matmul; reciprocal + gpsimd partition broadcast
  out = av * r + src_res -> DRAM
"""

import os
import sys

sys.path.insert(0, "/opt/trn_rl_repo")

import numpy as np
import ml_dtypes

BF16 = ml_dtypes.bfloat16

B, C, H, W = 4, 256, 64, 64
N = H * W            # 4096 keys (and queries per batch)
CQK = 32
NCORES = 8
QSH = N // 2         # 2048 queries per core
QC = 512             # q-chunk width (one PSUM bank)
NQC = QSH // QC      # 4 q-chunks
MT = 128             # m-tile (keys per scoresT tile)
NMT = N // MT        # 32 m-tiles
MG = int(os.environ.get("KERNEL_MG", "4"))   # m-tiles per exp group (<=4)
NG = NMT // MG       # groups per q-chunk
SBUFS = int(os.environ.get("KERNEL_SBUFS", "1"))
LOOP = int(os.environ.get("KERNEL_LOOP", "0"))  # >0: repeat body for timing

_last_results = None  # BassKernelResults of the most recent run (for test.py)


def _build_bass():
    import concourse.bass as bass
    import concourse.tile as tile
    from concourse import bacc, mybir
    from contextlib import ExitStack

    f32 = mybir.dt.float32
    bf16 = mybir.dt.bfloat16

    nc = bacc.Bacc("TRN2")

    # ---- DRAM I/O ----
    tgt_d = nc.dram_tensor("tgt", [C, N], bf16, kind="ExternalInput")
    srcq_d = nc.dram_tensor("srcq", [C, QSH], bf16, kind="ExternalInput")
    srcr_d = nc.dram_tensor("srcr", [C, QSH], f32, kind="ExternalInput")
    wqT_d = nc.dram_tensor("wqT", [C, CQK], bf16, kind="ExternalInput")
    wkT_d = nc.dram_tensor("wkT", [C, CQK], bf16, kind="ExternalInput")
    wvT_d = nc.dram_tensor("wvT", [C, C], bf16, kind="ExternalInput")
    bqk_d = nc.dram_tensor("bqk", [128, 2], f32, kind="ExternalInput")
    bv_d = nc.dram_tensor("bv", [1, C], bf16, kind="ExternalInput")
    out_d = nc.dram_tensor("out", [C, QSH], f32, kind="ExternalOutput")

    ones_col_d = nc.inline_tensor(np.ones((128, 1), dtype=BF16), name="ones_col")
    ones_row_d = nc.inline_tensor(np.ones((1, 128), dtype=BF16), name="ones_row")
    sel_np = np.zeros((128, 1), dtype=np.float32)
    for i in range(MG):
        sel_np[32 * i, 0] = 1.0
    sel_d = nc.inline_tensor(sel_np, name="sel_col")

    with tile.TileContext(nc) as tc:
        with (
            tc.tile_pool(name="const", bufs=1) as const,
            tc.tile_pool(name="data", bufs=1) as data,
        ):
            # ---- ACT table warmup: a dependency-free Exp so walrus's
            # inserted ACT_TABLE_LOAD lands on an instruction with <=1 wait.
            warm = const.tile([1, 8], f32, tag="warm")
            nc.vector.memset(warm, 0.0)
            nc.scalar.activation(
                out=warm, in_=warm, func=mybir.ActivationFunctionType.Exp
            )

            # ---- constants / weights ----
            wq0 = const.tile([128, CQK], bf16, tag="wq0")
            wq1 = const.tile([128, CQK], bf16, tag="wq1")
            wk0 = const.tile([128, CQK], bf16, tag="wk0")
            wk1 = const.tile([128, CQK], bf16, tag="wk1")
            wv0 = const.tile([128, C], bf16, tag="wv0")
            wv1 = const.tile([128, C], bf16, tag="wv1")
            bqk = const.tile([128, 2], f32, tag="bqk")
            bvr = const.tile([1, C], bf16, tag="bvr")
            ones_col = const.tile([128, 1], bf16, tag="ones_col")
            ones_row = const.tile([1, 128], bf16, tag="ones_row")
            sel_col = const.tile([128, 1], f32, tag="sel_col")
            nc.sync.dma_start(out=wq0, in_=wqT_d[0:128, :])
            nc.sync.dma_start(out=wq1, in_=wqT_d[128:256, :])
            nc.sync.dma_start(out=wk0, in_=wkT_d[0:128, :])
            nc.sync.dma_start(out=wk1, in_=wkT_d[128:256, :])
            nc.sync.dma_start(out=wv0, in_=wvT_d[0:128, :])
            nc.sync.dma_start(out=wv1, in_=wvT_d[128:256, :])
            nc.sync.dma_start(out=bqk, in_=bqk_d[:, :])
            nc.sync.dma_start(out=bvr, in_=bv_d[:, :])
            nc.sync.dma_start(out=ones_col, in_=ones_col_d[:, :])
            nc.sync.dma_start(out=ones_row, in_=ones_row_d[:, :])
            nc.sync.dma_start(out=sel_col, in_=sel_d[:, :])

            # ---- big data tiles ----
            tgt0 = data.tile([128, 8, 512], bf16, tag="tgt0")
            tgt1 = data.tile([128, 8, 512], bf16, tag="tgt1")
            for j in range(8):
                sl = slice(j * 512, (j + 1) * 512)
                nc.sync.dma_start(out=tgt0[:, j, :], in_=tgt_d[0:128, sl])
                nc.sync.dma_start(out=tgt1[:, j, :], in_=tgt_d[128:256, sl])
            srcq0 = data.tile([128, NQC, QC], bf16, tag="srcq0")
            srcq1 = data.tile([128, NQC, QC], bf16, tag="srcq1")
            srcr0 = data.tile([128, NQC, QC], f32, tag="srcr0")
            srcr1 = data.tile([128, NQC, QC], f32, tag="srcr1")
            for j in range(NQC):
                sl = slice(j * QC, (j + 1) * QC)
                nc.sync.dma_start(out=srcq0[:, j, :], in_=srcq_d[0:128, sl])
                nc.sync.dma_start(out=srcq1[:, j, :], in_=srcq_d[128:256, sl])
                nc.sync.dma_start(out=srcr0[:, j, :], in_=srcr_d[0:128, sl])
                nc.sync.dma_start(out=srcr1[:, j, :], in_=srcr_d[128:256, sl])

            # K4_sb[32*i + cqk, g, col] = K[cqk, (4g+i)*128 + col]
            # (4 m-tiles of a group live at partition blocks 0/32/64/96)
            K4_sb = data.tile([128, N // (4 * MT), MT], bf16, tag="K4_sb")
            # Q replicated at all 4 partition blocks
            Q4_sb = data.tile([128, NQC, QC], bf16, tag="Q4_sb")
            VT_sb = data.tile([128, NMT, C], bf16, tag="VT_sb")

            body_stack = ExitStack()
            if LOOP:
                body_stack.enter_context(tc.For_i(0, LOOP, 1))
            with body_stack:
                # ---- projections ----
                with (
                    tc.tile_pool(name="pv", bufs=3, space="PSUM") as pv,
                    tc.tile_pool(name="pk", bufs=2, space="PSUM") as pk,
                ):
                    # V^T tiles: VT[m,c] = sum_ch tgt[ch,m] WvT[ch,c]  (+ bv)
                    for mt in range(NMT):
                        ps = pv.tile([128, C], f32, tag="psv")
                        j, o = divmod(mt * MT, 512)
                        lhs0 = tgt0[:, j, o : o + MT]
                        lhs1 = tgt1[:, j, o : o + MT]
                        nc.tensor.matmul(ps, lhsT=lhs0, rhs=wv0, start=True, stop=False)
                        nc.tensor.matmul(ps, lhsT=lhs1, rhs=wv1, start=False, stop=False)
                        nc.tensor.matmul(
                            ps, lhsT=ones_row, rhs=bvr, start=False, stop=True
                        )
                        nc.vector.tensor_copy(out=VT_sb[:, mt, :], in_=ps)
                    # K: 4 m-tiles per group at partition blocks via col groups
                    for g in range(N // (4 * MT)):
                        ps = pk.tile([128, MT], f32, tag="psk")
                        for i in range(4):
                            mt = 4 * g + i
                            j, o = divmod(mt * MT, 512)
                            out_blk = ps[32 * i : 32 * (i + 1), :]
                            nc.tensor.matmul(
                                out_blk, lhsT=wk0, rhs=tgt0[:, j, o : o + MT],
                                start=True, stop=False, tile_position=(0, 32 * i),
                            )
                            nc.tensor.matmul(
                                out_blk, lhsT=wk1, rhs=tgt1[:, j, o : o + MT],
                                start=False, stop=True, tile_position=(0, 32 * i),
                            )
                        nc.vector.tensor_scalar_add(K4_sb[:, g, :], ps, bqk[:, 1:2])
                    # Q: proj into partition block 0, then replicate via DMA
                    for j in range(NQC):
                        ps = pk.tile([CQK, QC], f32, tag="psq")
                        nc.tensor.matmul(
                            ps, lhsT=wq0, rhs=srcq0[:, j, :], start=True, stop=False
                        )
                        nc.tensor.matmul(
                            ps, lhsT=wq1, rhs=srcq1[:, j, :], start=False, stop=True
                        )
                        nc.vector.tensor_scalar_add(
                            Q4_sb[0:CQK, j, :], ps, bqk[0:CQK, 0:1]
                        )
                    for i in range(1, 4):
                        nc.sync.dma_start(
                            out=Q4_sb[32 * i : 32 * (i + 1), :, :],
                            in_=Q4_sb[0:CQK, :, :],
                        )

                # ---- attention ----
                with (
                    tc.tile_pool(name="ps_s", bufs=SBUFS, space="PSUM") as ps_s,
                    tc.tile_pool(name="ps_av", bufs=1, space="PSUM") as ps_av,
                    tc.tile_pool(name="ps_l", bufs=1, space="PSUM") as ps_l,
                    tc.tile_pool(name="att", bufs=3) as att,
                    tc.tile_pool(name="outp", bufs=4) as outp,
                ):
                    for qc in range(NQC):
                        av0 = ps_av.tile([128, QC], f32, tag="av0")
                        av1 = ps_av.tile([128, QC], f32, tag="av1")
                        lrow4 = ps_l.tile([128, QC], f32, tag="lrow4")
                        l_ps = ps_l.tile([1, QC], f32, tag="l_ps")
                        nc.vector.memset(lrow4, 0.0)
                        for g in range(NG):
                            S = ps_s.tile([128, MG, QC], f32, tag="S")
                            for i in range(MG):
                                mt = g * MG + i
                                gg, ii = divmod(mt, 4)
                                blk = slice(32 * ii, 32 * (ii + 1))
                                nc.tensor.matmul(
                                    S[:, i, :],
                                    lhsT=K4_sb[blk, gg, :],
                                    rhs=Q4_sb[blk, qc, :],
                                    start=True,
                                    stop=True,
                                    tile_position=(32 * ii, 0),
                                )
                            expT = att.tile([128, MG, QC], bf16, tag="expT")
                            nc.scalar.activation(
                                out=expT.rearrange("p a b -> p (a b)"),
                                in_=S.rearrange("p a b -> p (a b)"),
                                func=mybir.ActivationFunctionType.Exp,
                            )
                            for i in range(MG):
                                mt = g * MG + i
                                first = mt == 0
                                last = mt == NMT - 1
                                nc.tensor.matmul(
                                    av0,
                                    lhsT=VT_sb[:, mt, 0:128],
                                    rhs=expT[:, i, :],
                                    start=first,
                                    stop=last,
                                )
                                nc.tensor.matmul(
                                    av1,
                                    lhsT=VT_sb[:, mt, 128:256],
                                    rhs=expT[:, i, :],
                                    start=first,
                                    stop=last,
                                )
                                # denominator partial at partition 32*i
                                nc.tensor.matmul(
                                    lrow4[32 * i : 32 * i + 1, :],
                                    lhsT=ones_col,
                                    rhs=expT[:, i, :],
                                    start=first,
                                    stop=last,
                                    tile_position=(0, 32 * i),
                                )
                        # fold 4 partial rows -> l, reciprocal, broadcast
                        lrow4_sb = outp.tile([128, QC], f32, tag="lrow4_sb")
                        nc.vector.tensor_copy(out=lrow4_sb, in_=lrow4)
                        nc.tensor.matmul(
                            l_ps, lhsT=sel_col, rhs=lrow4_sb, start=True, stop=True
                        )
                        l_sb = outp.tile([1, QC], f32, tag="l_sb")
                        r_sb = outp.tile([1, QC], f32, tag="r_sb")
                        r_rep = outp.tile([128, QC], f32, tag="r_rep")
                        nc.vector.tensor_copy(out=l_sb, in_=l_ps)
                        nc.vector.reciprocal_approx_fast(out=r_sb, in_=l_sb)
                        nc.gpsimd.partition_broadcast(r_rep, r_sb)
                        for ci, (av, srcr) in enumerate(((av0, srcr0), (av1, srcr1))):
                            o = outp.tile([128, QC], f32, tag=f"o{ci}")
                            nc.vector.tensor_mul(o, av, r_rep)
                            nc.vector.tensor_add(o, o, srcr[:, qc, :])
                            nc.sync.dma_start(
                                out=out_d[
                                    128 * ci : 128 * (ci + 1), qc * QC : (qc + 1) * QC
                                ],
                                in_=o,
                            )
    nc.compile()
    return nc


_cached = None


def _get_bass():
    global _cached
    if _cached is None:
        _cached = _build_bass()
    return _cached


def kernel(src_feat, tgt_feat, Wq, bq, Wk, bk, Wv, bv):
    """Full inputs in, full output out. Shards internally across 8 cores."""
    global _last_results
    from concourse.bass_utils import run_bass_kernel_spmd

    src = np.asarray(src_feat, dtype=np.float32).reshape(B, C, N)
    tgt = np.asarray(tgt_feat, dtype=np.float32).reshape(B, C, N)
    wqT = np.ascontiguousarray(np.asarray(Wq, np.float32).T).astype(BF16)
    wkT = np.ascontiguousarray(np.asarray(Wk, np.float32).T).astype(BF16)
    wvT = np.ascontiguousarray(np.asarray(Wv, np.float32).T).astype(BF16)
    bqk = np.tile(
        np.stack([np.asarray(bq, np.float32), np.asarray(bk, np.float32)], axis=1),
        (4, 1),
    )  # [128, 2]
    bvr = np.asarray(bv, np.float32).reshape(1, C).astype(BF16)

    tgt_bf = tgt.astype(BF16)

    in_maps = []
    for c in range(NCORES):
        b, h = divmod(c, 2)
        qsl = slice(h * QSH, (h + 1) * QSH)
        in_maps.append(
            {
                "tgt": np.ascontiguousarray(tgt_bf[b]),
                "srcq": np.ascontiguousarray(src[b, :, qsl]).astype(BF16),
                "srcr": np.ascontiguousarray(src[b, :, qsl]),
                "wqT": wqT,
                "wkT": wkT,
                "wvT": wvT,
                "bqk": np.ascontiguousarray(bqk),
                "bv": bvr,
            }
        )

    nc = _get_bass()
    res = run_bass_kernel_spmd(
        nc,
        in_maps,
        core_ids=list(range(NCORES)),
        trace=bool(int(os.environ.get("KERNEL_TRACE", "0"))),
    )
    _last_results = res

    out = np.empty((B, C, N), dtype=np.float32)
    for c in range(NCORES):
        b, h = divmod(c, 2)
        out[b, :, h * QSH : (h + 1) * QSH] = res.results[c]["out"]
    return out.reshape(B, C, H, W)


# revision 26
# speedup vs baseline: 6627.8561x; 6481.6306x over previous
"""CrossViewTransformer kernel for 8 Trainium2 NeuronCores.

Problem: B=4, C=256, H=W=64 (N=4096), Cqk=32 cross-attention + residual.
  Q = Wq@src, K = Wk@tgt, V = Wv@tgt  (1x1 convs over channels)
  out = softmax(Q^T K) @ V^T + src     (no 1/sqrt(d) scale)

Sharding: 8 cores = 4 batches x 2 query-halves. Each core computes attention
for 2048 queries x 4096 keys of one batch. The V projection is replicated
across the 2 cores of a batch (cheap: 0.5 GFLOP) while the expensive parts
(scores, exp, attn@V) are fully split.

Per-core pipeline (per 512-query chunk, m = key index, groups of MG=2
m-tiles):
  scoresT[m,q]: row-packed matmuls (K=32 contraction at row groups 32*i)
    into a double-buffered PSUM tile; one Exp activation per group writes
    fp8e4m3 attention weights to SBUF (no max-subtraction needed: scores
    are O(1) by construction, exp never overflows)
  attn@V: fp8 DoubleRow matmuls (pairs of m-tiles per instruction)
    accumulating [c_half, q] in PSUM across all 32 m-tiles
  denominator: col-packed ones-matmuls -> rows {0,32} of one PSUM bank;
    folded+broadcast by one gpsimd partition_all_reduce, then a custom-DVE
    approximate reciprocal gives 1/l replicated on all partitions
  out = av * r + src_res -> DRAM  (residual dominates the output, which is
    why fp8 attention weights cost only ~2e-4 relative error)
"""

import os
import sys

sys.path.insert(0, "/opt/trn_rl_repo")

import numpy as np
import ml_dtypes

BF16 = ml_dtypes.bfloat16

B, C, H, W = 4, 256, 64, 64
N = H * W            # 4096 keys (and queries per batch)
CQK = 32
NCORES = 8
QSH = N // 2         # 2048 queries per core
QC = 512             # q-chunk width (one PSUM bank)
NQC = QSH // QC      # 4 q-chunks
MT = 128             # m-tile (keys per scoresT tile)
NMT = N // MT        # 32 m-tiles
MG = int(os.environ.get("KERNEL_MG", "2"))   # m-tiles per exp group (<=4)
NG = NMT // MG       # groups per q-chunk
SBUFS = int(os.environ.get("KERNEL_SBUFS", "2"))
LOOP = int(os.environ.get("KERNEL_LOOP", "0"))  # >0: repeat body for timing
# timing bisection: 0=empty loop body, 1=+proj, 2=+QK, 3=+exp, 4=+AV,
# 5=+L matmuls, 6=full
STAGE = int(os.environ.get("KERNEL_STAGE", "6"))
# split exp into 2 half-group ops so QK(g+1) overlaps exp(g) in one S tile
EXPSPLIT = bool(int(os.environ.get("KERNEL_EXPSPLIT", "0")))
# fp8e4m3 attn weights + V with DoubleRow attn@V matmuls (2x PE rate)
FP8AV = bool(int(os.environ.get("KERNEL_FP8AV", "1")))

_last_results = None  # BassKernelResults of the most recent run (for test.py)


def _build_bass():
    import concourse.bass as bass
    import concourse.tile as tile
    from concourse import bacc, mybir
    from concourse import bass_isa
    from contextlib import ExitStack

    f32 = mybir.dt.float32
    bf16 = mybir.dt.bfloat16
    fp8 = mybir.dt.float8e4
    at_dt = fp8 if FP8AV else bf16  # attn-weight / V dtype
    AT_NP = ml_dtypes.float8_e4m3 if FP8AV else BF16

    nc = bacc.Bacc("TRN2")

    # ---- DRAM I/O ----
    tgt_d = nc.dram_tensor("tgt", [C, N], bf16, kind="ExternalInput")
    srcq_d = nc.dram_tensor("srcq", [C, QSH], bf16, kind="ExternalInput")
    srcr_d = nc.dram_tensor("srcr", [C, QSH], f32, kind="ExternalInput")
    wqT_d = nc.dram_tensor("wqT", [C, CQK], bf16, kind="ExternalInput")
    wkT_d = nc.dram_tensor("wkT", [C, CQK], bf16, kind="ExternalInput")
    wvT_d = nc.dram_tensor("wvT", [C, C], bf16, kind="ExternalInput")
    bqk_d = nc.dram_tensor("bqk", [128, 2], f32, kind="ExternalInput")
    bv_d = nc.dram_tensor("bv", [1, C], bf16, kind="ExternalInput")
    out_d = nc.dram_tensor("out", [C, QSH], f32, kind="ExternalOutput")

    ones_col_d = nc.inline_tensor(np.ones((128, 1), dtype=AT_NP), name="ones_col")
    ones_row_d = nc.inline_tensor(np.ones((1, 128), dtype=BF16), name="ones_row")

    with tile.TileContext(nc) as tc:
        with (
            tc.tile_pool(name="const", bufs=1) as const,
            tc.tile_pool(name="data", bufs=1) as data,
        ):
            # ---- ACT table warmup: a dependency-free Exp so walrus's
            # inserted ACT_TABLE_LOAD lands on an instruction with <=1 wait.
            warm = const.tile([1, 8], f32, tag="warm")
            nc.vector.memset(warm, 0.0)
            nc.scalar.activation(
                out=warm, in_=warm, func=mybir.ActivationFunctionType.Exp
            )

            # ---- constants / weights ----
            wq0 = const.tile([128, CQK], bf16, tag="wq0")
            wq1 = const.tile([128, CQK], bf16, tag="wq1")
            wk0 = const.tile([128, CQK], bf16, tag="wk0")
            wk1 = const.tile([128, CQK], bf16, tag="wk1")
            wv0 = const.tile([128, C], bf16, tag="wv0")
            wv1 = const.tile([128, C], bf16, tag="wv1")
            bqk = const.tile([128, 2], f32, tag="bqk")
            bvr = const.tile([1, C], bf16, tag="bvr")
            ones_col = const.tile([128, 1], at_dt, tag="ones_col")
            ones_row = const.tile([1, 128], bf16, tag="ones_row")
            nc.sync.dma_start(out=wq0, in_=wqT_d[0:128, :])
            nc.sync.dma_start(out=wq1, in_=wqT_d[128:256, :])
            nc.sync.dma_start(out=wk0, in_=wkT_d[0:128, :])
            nc.sync.dma_start(out=wk1, in_=wkT_d[128:256, :])
            nc.sync.dma_start(out=wv0, in_=wvT_d[0:128, :])
            nc.sync.dma_start(out=wv1, in_=wvT_d[128:256, :])
            nc.sync.dma_start(out=bqk, in_=bqk_d[:, :])
            nc.sync.dma_start(out=bvr, in_=bv_d[:, :])
            nc.sync.dma_start(out=ones_col, in_=ones_col_d[:, :])
            nc.sync.dma_start(out=ones_row, in_=ones_row_d[:, :])

            # ---- big data tiles ----
            tgt0 = data.tile([128, 8, 512], bf16, tag="tgt0")
            tgt1 = data.tile([128, 8, 512], bf16, tag="tgt1")
            for j in range(8):
                sl = slice(j * 512, (j + 1) * 512)
                nc.sync.dma_start(out=tgt0[:, j, :], in_=tgt_d[0:128, sl])
                nc.sync.dma_start(out=tgt1[:, j, :], in_=tgt_d[128:256, sl])
            srcq0 = data.tile([128, NQC, QC], bf16, tag="srcq0")
            srcq1 = data.tile([128, NQC, QC], bf16, tag="srcq1")
            srcr0 = data.tile([128, NQC, QC], f32, tag="srcr0")
            srcr1 = data.tile([128, NQC, QC], f32, tag="srcr1")
            for j in range(NQC):
                sl = slice(j * QC, (j + 1) * QC)
                nc.sync.dma_start(out=srcq0[:, j, :], in_=srcq_d[0:128, sl])
                nc.sync.dma_start(out=srcq1[:, j, :], in_=srcq_d[128:256, sl])
                nc.sync.dma_start(out=srcr0[:, j, :], in_=srcr_d[0:128, sl])
                nc.sync.dma_start(out=srcr1[:, j, :], in_=srcr_d[128:256, sl])

            # persistent staging tile for the denominator fold: rows 0/32
            # carry the two col-group partials, the rest stay zero
            lrow_sb = data.tile([128, QC], f32, tag="lrow_sb")
            nc.vector.memset(lrow_sb, 0.0)

            # bv broadcast to all partitions once (for the VT copy+bias add)
            bv_rep = data.tile([128, C], bf16, tag="bv_rep")
            nc.gpsimd.partition_broadcast(bv_rep, bvr)

            # K4_sb[32*i + cqk, g, col] = K[cqk, (4g+i)*128 + col]
            # (4 m-tiles of a group live at partition blocks 0/32/64/96)
            K4_sb = data.tile([128, N // (4 * MT), MT], bf16, tag="K4_sb")
            # Q replicated at all 4 partition blocks
            Q4_sb = data.tile([128, NQC, QC], bf16, tag="Q4_sb")
            VT_sb = data.tile([128, NMT, C], at_dt, tag="VT_sb")

            body_stack = ExitStack()
            if LOOP:
                body_stack.enter_context(tc.For_i(0, LOOP, 1))
            with body_stack:
                if STAGE == 0:
                    tick = data.tile([1, 8], f32, tag="tick")
                    nc.vector.memset(tick, 1.0)
                # ---- projections ----
                if STAGE >= 1:
                    with (
                        tc.tile_pool(name="pv", bufs=3, space="PSUM") as pv,
                        tc.tile_pool(name="pk", bufs=2, space="PSUM") as pk,
                    ):
                        # V^T tiles: VT[m,c] = sum_ch tgt[ch,m] WvT[ch,c] (+ bv)
                        for mt in range(NMT):
                            ps = pv.tile([128, C], f32, tag="psv")
                            j, o = divmod(mt * MT, 512)
                            lhs0 = tgt0[:, j, o : o + MT]
                            lhs1 = tgt1[:, j, o : o + MT]
                            nc.tensor.matmul(
                                ps, lhsT=lhs0, rhs=wv0, start=True, stop=False
                            )
                            nc.tensor.matmul(
                                ps, lhsT=lhs1, rhs=wv1, start=False, stop=True
                            )
                            nc.vector.tensor_add(VT_sb[:, mt, :], ps, bv_rep)
                        # K: 4 m-tiles per group at partition blocks (col groups)
                        for g in range(N // (4 * MT)):
                            ps = pk.tile([128, MT], f32, tag="psk")
                            for i in range(4):
                                mt = 4 * g + i
                                j, o = divmod(mt * MT, 512)
                                out_blk = ps[32 * i : 32 * (i + 1), :]
                                nc.tensor.matmul(
                                    out_blk, lhsT=wk0, rhs=tgt0[:, j, o : o + MT],
                                    start=True, stop=False, tile_position=(0, 32 * i),
                                )
                                nc.tensor.matmul(
                                    out_blk, lhsT=wk1, rhs=tgt1[:, j, o : o + MT],
                                    start=False, stop=True, tile_position=(0, 32 * i),
                                )
                            nc.vector.tensor_scalar_add(K4_sb[:, g, :], ps, bqk[:, 1:2])
                        # Q: proj into partition block 0, then replicate via DMA
                        for j in range(NQC):
                            ps = pk.tile([CQK, QC], f32, tag="psq")
                            nc.tensor.matmul(
                                ps, lhsT=wq0, rhs=srcq0[:, j, :], start=True, stop=False
                            )
                            nc.tensor.matmul(
                                ps, lhsT=wq1, rhs=srcq1[:, j, :], start=False, stop=True
                            )
                            nc.vector.tensor_scalar_add(
                                Q4_sb[0:CQK, j, :], ps, bqk[0:CQK, 0:1]
                            )
                        for i in range(1, 4):
                            nc.sync.dma_start(
                                out=Q4_sb[32 * i : 32 * (i + 1), :, :],
                                in_=Q4_sb[0:CQK, :, :],
                            )

                # ---- attention ----
                if STAGE >= 2:
                    with (
                        tc.tile_pool(name="ps_s", bufs=SBUFS, space="PSUM") as ps_s,
                        tc.tile_pool(name="ps_av", bufs=1, space="PSUM") as ps_av,
                        tc.tile_pool(name="ps_l", bufs=1, space="PSUM") as ps_l,
                        tc.tile_pool(name="att", bufs=3) as att,
                        tc.tile_pool(name="outp", bufs=4) as outp,
                    ):
                        for qc in range(NQC):
                            av0 = ps_av.tile([128, QC], f32, tag="av0")
                            av1 = ps_av.tile([128, QC], f32, tag="av1")
                            lrow = ps_l.tile([128, QC], f32, tag="lrow")
                            for g in range(NG):
                                S = ps_s.tile([128, MG, QC], f32, tag="S")
                                for i in range(MG):
                                    mt = g * MG + i
                                    gg, ii = divmod(mt, 4)
                                    blk = slice(32 * ii, 32 * (ii + 1))
                                    nc.tensor.matmul(
                                        S[:, i, :],
                                        lhsT=K4_sb[blk, gg, :],
                                        rhs=Q4_sb[blk, qc, :],
                                        start=True,
                                        stop=True,
                                        tile_position=(32 * ii, 0),
                                    )
                                expT = att.tile([128, MG, QC], at_dt, tag="expT")
                                if STAGE >= 3 and EXPSPLIT:
                                    h = MG // 2
                                    for e in range(2):
                                        nc.scalar.activation(
                                            out=expT[:, e * h : (e + 1) * h, :].rearrange(
                                                "p a b -> p (a b)"
                                            ),
                                            in_=S[:, e * h : (e + 1) * h, :].rearrange(
                                                "p a b -> p (a b)"
                                            ),
                                            func=mybir.ActivationFunctionType.Exp,
                                        )
                                elif STAGE >= 3:
                                    nc.scalar.activation(
                                        out=expT.rearrange("p a b -> p (a b)"),
                                        in_=S.rearrange("p a b -> p (a b)"),
                                        func=mybir.ActivationFunctionType.Exp,
                                    )
                                if STAGE >= 4 and FP8AV:
                                    for t in range(MG // 2):
                                        mt0 = g * MG + 2 * t
                                        first = mt0 == 0
                                        last = mt0 == NMT - 2
                                        for av, cs in ((av0, slice(0, 128)), (av1, slice(128, 256))):
                                            nc.tensor.matmul(
                                                av,
                                                lhsT=VT_sb[:, mt0 : mt0 + 2, cs],
                                                rhs=expT[:, 2 * t : 2 * t + 2, :],
                                                start=first,
                                                stop=last,
                                                perf_mode=mybir.MatmulPerfMode.DoubleRow,
                                            )
                                if STAGE >= 4 and not FP8AV:
                                    for i in range(MG):
                                        mt = g * MG + i
                                        first = mt == 0
                                        last = mt == NMT - 1
                                        nc.tensor.matmul(
                                            av0,
                                            lhsT=VT_sb[:, mt, 0:128],
                                            rhs=expT[:, i, :],
                                            start=first,
                                            stop=last,
                                        )
                                        nc.tensor.matmul(
                                            av1,
                                            lhsT=VT_sb[:, mt, 128:256],
                                            rhs=expT[:, i, :],
                                            start=first,
                                            stop=last,
                                        )
                                if STAGE >= 5:
                                    # denominator partials: adjacent col-packed
                                    # matmuls -> rows 32*i of one PSUM bank
                                    for i in range(MG):
                                        nc.tensor.matmul(
                                            lrow[32 * i : 32 * i + 1, :],
                                            lhsT=ones_col,
                                            rhs=expT[:, i, :],
                                            start=g == 0,
                                            stop=g == NG - 1,
                                            tile_position=(0, 32 * i),
                                        )
                            if STAGE < 6:
                                continue
                            # free the av/lrow banks fast: copy to SBUF, then
                            # run the fold/recip/normalize tail asynchronously
                            av0_sb = outp.tile([128, QC], f32, tag="av0_sb")
                            av1_sb = outp.tile([128, QC], f32, tag="av1_sb")
                            nc.vector.tensor_copy(out=av0_sb, in_=av0)
                            nc.vector.tensor_copy(out=av1_sb, in_=av1)
                            for i in range(MG):
                                nc.vector.tensor_copy(
                                    out=lrow_sb[32 * i : 32 * i + 1, :],
                                    in_=lrow[32 * i : 32 * i + 1, :],
                                )
                            l_rep = outp.tile([128, QC], f32, tag="l_rep")
                            r_rep = outp.tile([128, QC], f32, tag="r_rep")
                            nc.gpsimd.partition_all_reduce(
                                l_rep, lrow_sb, 128, bass_isa.ReduceOp.add
                            )
                            nc.vector.reciprocal_approx_fast(out=r_rep, in_=l_rep)
                            for ci, (av_sb, srcr) in enumerate(
                                ((av0_sb, srcr0), (av1_sb, srcr1))
                            ):
                                o = outp.tile([128, QC], f32, tag=f"o{ci}")
                                nc.vector.tensor_mul(o, av_sb, r_rep)
                                nc.vector.tensor_add(o, o, srcr[:, qc, :])
                                nc.sync.dma_start(
                                    out=out_d[
                                        128 * ci : 128 * (ci + 1),
                                        qc * QC : (qc + 1) * QC,
                                    ],
                                    in_=o,
                                )
    nc.compile()
    return nc


_cached = None


def _get_bass():
    global _cached
    if _cached is None:
        _cached = _build_bass()
    return _cached


def kernel(src_feat, tgt_feat, Wq, bq, Wk, bk, Wv, bv):
    """Full inputs in, full output out. Shards internally across 8 cores."""
    global _last_results
    from concourse.bass_utils import run_bass_kernel_spmd

    src = np.asarray(src_feat, dtype=np.float32).reshape(B, C, N)
    tgt = np.asarray(tgt_feat, dtype=np.float32).reshape(B, C, N)
    wqT = np.ascontiguousarray(np.asarray(Wq, np.float32).T).astype(BF16)
    wkT = np.ascontiguousarray(np.asarray(Wk, np.float32).T).astype(BF16)
    wvT = np.ascontiguousarray(np.asarray(Wv, np.float32).T).astype(BF16)
    bqk = np.tile(
        np.stack([np.asarray(bq, np.float32), np.asarray(bk, np.float32)], axis=1),
        (4, 1),
    )  # [128, 2]
    bvr = np.asarray(bv, np.float32).reshape(1, C).astype(BF16)

    tgt_bf = tgt.astype(BF16)

    in_maps = []
    for c in range(NCORES):
        b, h = divmod(c, 2)
        qsl = slice(h * QSH, (h + 1) * QSH)
        in_maps.append(
            {
                "tgt": np.ascontiguousarray(tgt_bf[b]),
                "srcq": np.ascontiguousarray(src[b, :, qsl]).astype(BF16),
                "srcr": np.ascontiguousarray(src[b, :, qsl]),
                "wqT": wqT,
                "wkT": wkT,
                "wvT": wvT,
                "bqk": np.ascontiguousarray(bqk),
                "bv": bvr,
            }
        )

    nc = _get_bass()
    res = run_bass_kernel_spmd(
        nc,
        in_maps,
        core_ids=list(range(NCORES)),
        trace=bool(int(os.environ.get("KERNEL_TRACE", "0"))),
    )
    _last_results = res

    out = np.empty((B, C, N), dtype=np.float32)
    for c in range(NCORES):
        b, h = divmod(c, 2)
        out[b, :, h * QSH : (h + 1) * QSH] = res.results[c]["out"]
    return out.reshape(B, C, H, W)


# revision 27
# speedup vs baseline: 19447.6703x; 2.9342x over previous
"""CrossViewTransformer kernel for 8 Trainium2 NeuronCores.

Problem: B=4, C=256, H=W=64 (N=4096), Cqk=32 cross-attention + residual.
  Q = Wq@src, K = Wk@tgt, V = Wv@tgt  (1x1 convs over channels)
  out = softmax(Q^T K) @ V^T + src     (no 1/sqrt(d) scale)

Sharding: 8 cores = 4 batches x 2 query-halves. Each core computes attention
for 2048 queries x 4096 keys of one batch. The V projection is replicated
across the 2 cores of a batch (cheap: 0.5 GFLOP) while the expensive parts
(scores, exp, attn@V) are fully split.

Per-core pipeline (per 512-query chunk, m = key index, groups of MG=2
m-tiles):
  scoresT[m,q]: row-packed matmuls (K=32 contraction at row groups 32*i)
    into a double-buffered PSUM tile; one Exp activation per group writes
    fp8e4m3 attention weights to SBUF (no max-subtraction needed: scores
    are O(1) by construction, exp never overflows)
  attn@V: fp8 DoubleRow matmuls (pairs of m-tiles per instruction)
    accumulating [c_half, q] in PSUM across all 32 m-tiles
  denominator: col-packed ones-matmuls -> rows {0,32} of one PSUM bank;
    folded+broadcast by one gpsimd partition_all_reduce, then a custom-DVE
    approximate reciprocal gives 1/l replicated on all partitions
  out = av * r + src_res -> DRAM  (residual dominates the output, which is
    why fp8 attention weights cost only ~2e-4 relative error)
"""

import os
import sys

sys.path.insert(0, "/opt/trn_rl_repo")

import numpy as np
import ml_dtypes

BF16 = ml_dtypes.bfloat16

B, C, H, W = 4, 256, 64, 64
N = H * W            # 4096 keys (and queries per batch)
CQK = 32
NCORES = 8
QSH = N // 2         # 2048 queries per core
QC = 512             # q-chunk width (one PSUM bank)
NQC = QSH // QC      # 4 q-chunks
MT = 128             # m-tile (keys per scoresT tile)
NMT = N // MT        # 32 m-tiles
MG = int(os.environ.get("KERNEL_MG", "2"))   # m-tiles per exp group (<=4)
NG = NMT // MG       # groups per q-chunk
SBUFS = int(os.environ.get("KERNEL_SBUFS", "2"))
LOOP = int(os.environ.get("KERNEL_LOOP", "0"))  # >0: repeat body for timing
# timing bisection: 0=empty loop body, 1=+proj, 2=+QK, 3=+exp, 4=+AV,
# 5=+L matmuls, 6=full
STAGE = int(os.environ.get("KERNEL_STAGE", "6"))
# split exp into 2 half-group ops so QK(g+1) overlaps exp(g) in one S tile
EXPSPLIT = bool(int(os.environ.get("KERNEL_EXPSPLIT", "0")))
# fp8e4m3 attn weights + V with DoubleRow attn@V matmuls (2x PE rate)
FP8AV = bool(int(os.environ.get("KERNEL_FP8AV", "1")))

_last_results = None  # BassKernelResults of the most recent run (for test.py)


def _build_bass():
    import concourse.bass as bass
    import concourse.tile as tile
    from concourse import bacc, mybir
    from concourse import bass_isa
    from contextlib import ExitStack

    f32 = mybir.dt.float32
    bf16 = mybir.dt.bfloat16
    fp8 = mybir.dt.float8e4
    at_dt = fp8 if FP8AV else bf16  # attn-weight / V dtype
    AT_NP = ml_dtypes.float8_e4m3 if FP8AV else BF16

    nc = bacc.Bacc("TRN2")

    # ---- DRAM I/O ----
    tgt_d = nc.dram_tensor("tgt", [C, N], bf16, kind="ExternalInput")
    srcq_d = nc.dram_tensor("srcq", [C, QSH], bf16, kind="ExternalInput")
    srcr_d = nc.dram_tensor("srcr", [C, QSH], f32, kind="ExternalInput")
    wqT_d = nc.dram_tensor("wqT", [C, CQK], bf16, kind="ExternalInput")
    wkT_d = nc.dram_tensor("wkT", [C, CQK], bf16, kind="ExternalInput")
    wvT_d = nc.dram_tensor("wvT", [C, C], bf16, kind="ExternalInput")
    bqk_d = nc.dram_tensor("bqk", [128, 2], f32, kind="ExternalInput")
    bv_d = nc.dram_tensor("bv", [1, C], bf16, kind="ExternalInput")
    out_d = nc.dram_tensor("out", [C, QSH], f32, kind="ExternalOutput")

    ones_col_d = nc.inline_tensor(np.ones((128, 1), dtype=AT_NP), name="ones_col")
    ones_row_d = nc.inline_tensor(np.ones((1, 128), dtype=BF16), name="ones_row")

    with tile.TileContext(nc) as tc:
        with (
            tc.tile_pool(name="const", bufs=1) as const,
            tc.tile_pool(name="data", bufs=1) as data,
        ):
            # ---- ACT table warmup: a dependency-free Exp so walrus's
            # inserted ACT_TABLE_LOAD lands on an instruction with <=1 wait.
            warm = const.tile([1, 8], f32, tag="warm")
            nc.vector.memset(warm, 0.0)
            nc.scalar.activation(
                out=warm, in_=warm, func=mybir.ActivationFunctionType.Exp
            )

            # ---- constants / weights ----
            wq0 = const.tile([128, CQK], bf16, tag="wq0")
            wq1 = const.tile([128, CQK], bf16, tag="wq1")
            wk0 = const.tile([128, CQK], bf16, tag="wk0")
            wk1 = const.tile([128, CQK], bf16, tag="wk1")
            wv0 = const.tile([128, C], bf16, tag="wv0")
            wv1 = const.tile([128, C], bf16, tag="wv1")
            bqk = const.tile([128, 2], f32, tag="bqk")
            bvr = const.tile([1, C], bf16, tag="bvr")
            ones_col = const.tile([128, 1], at_dt, tag="ones_col")
            ones_row = const.tile([1, 128], bf16, tag="ones_row")
            nc.sync.dma_start(out=wq0, in_=wqT_d[0:128, :])
            nc.sync.dma_start(out=wq1, in_=wqT_d[128:256, :])
            nc.sync.dma_start(out=wk0, in_=wkT_d[0:128, :])
            nc.sync.dma_start(out=wk1, in_=wkT_d[128:256, :])
            nc.sync.dma_start(out=wv0, in_=wvT_d[0:128, :])
            nc.sync.dma_start(out=wv1, in_=wvT_d[128:256, :])
            nc.sync.dma_start(out=bqk, in_=bqk_d[:, :])
            nc.sync.dma_start(out=bvr, in_=bv_d[:, :])
            nc.sync.dma_start(out=ones_col, in_=ones_col_d[:, :])
            nc.sync.dma_start(out=ones_row, in_=ones_row_d[:, :])

            # ---- big data tiles ----
            tgt0 = data.tile([128, 8, 512], bf16, tag="tgt0")
            tgt1 = data.tile([128, 8, 512], bf16, tag="tgt1")
            for j in range(8):
                sl = slice(j * 512, (j + 1) * 512)
                nc.sync.dma_start(out=tgt0[:, j, :], in_=tgt_d[0:128, sl])
                nc.sync.dma_start(out=tgt1[:, j, :], in_=tgt_d[128:256, sl])
            srcq0 = data.tile([128, NQC, QC], bf16, tag="srcq0")
            srcq1 = data.tile([128, NQC, QC], bf16, tag="srcq1")
            srcr0 = data.tile([128, NQC, QC], f32, tag="srcr0")
            srcr1 = data.tile([128, NQC, QC], f32, tag="srcr1")
            for j in range(NQC):
                sl = slice(j * QC, (j + 1) * QC)
                nc.sync.dma_start(out=srcq0[:, j, :], in_=srcq_d[0:128, sl])
                nc.sync.dma_start(out=srcq1[:, j, :], in_=srcq_d[128:256, sl])
                nc.sync.dma_start(out=srcr0[:, j, :], in_=srcr_d[0:128, sl])
                nc.sync.dma_start(out=srcr1[:, j, :], in_=srcr_d[128:256, sl])

            # persistent staging tile for the denominator fold: rows 0/32
            # carry the two col-group partials, the rest stay zero
            lrow_sb = data.tile([128, QC], f32, tag="lrow_sb")
            nc.vector.memset(lrow_sb, 0.0)

            # bv broadcast to all partitions once (for the VT copy+bias add)
            bv_rep = data.tile([128, C], bf16, tag="bv_rep")
            nc.gpsimd.partition_broadcast(bv_rep, bvr)

            # K4_sb[32*i + cqk, g, col] = K[cqk, (4g+i)*128 + col]
            # (4 m-tiles of a group live at partition blocks 0/32/64/96)
            K4_sb = data.tile([128, N // (4 * MT), MT], bf16, tag="K4_sb")
            # Q replicated at all 4 partition blocks
            Q4_sb = data.tile([128, NQC, QC], bf16, tag="Q4_sb")
            VT_sb = data.tile([128, NMT, C], at_dt, tag="VT_sb")

            body_stack = ExitStack()
            if LOOP:
                body_stack.enter_context(tc.For_i(0, LOOP, 1))
            with body_stack:
                if STAGE == 0:
                    tick = data.tile([1, 8], f32, tag="tick")
                    nc.vector.memset(tick, 1.0)
                # ---- projections ----
                if STAGE >= 1:
                    with (
                        tc.tile_pool(name="pv", bufs=3, space="PSUM") as pv,
                        tc.tile_pool(name="pk", bufs=2, space="PSUM") as pk,
                    ):
                        # V^T tiles: VT[m,c] = sum_ch tgt[ch,m] WvT[ch,c] (+ bv)
                        for mt in range(NMT):
                            ps = pv.tile([128, C], f32, tag="psv")
                            j, o = divmod(mt * MT, 512)
                            lhs0 = tgt0[:, j, o : o + MT]
                            lhs1 = tgt1[:, j, o : o + MT]
                            nc.tensor.matmul(
                                ps, lhsT=lhs0, rhs=wv0, start=True, stop=False
                            )
                            nc.tensor.matmul(
                                ps, lhsT=lhs1, rhs=wv1, start=False, stop=True
                            )
                            nc.vector.tensor_add(VT_sb[:, mt, :], ps, bv_rep)
                        # K: 4 m-tiles per group at partition blocks (col groups)
                        for g in range(N // (4 * MT)):
                            ps = pk.tile([128, MT], f32, tag="psk")
                            for i in range(4):
                                mt = 4 * g + i
                                j, o = divmod(mt * MT, 512)
                                out_blk = ps[32 * i : 32 * (i + 1), :]
                                nc.tensor.matmul(
                                    out_blk, lhsT=wk0, rhs=tgt0[:, j, o : o + MT],
                                    start=True, stop=False, tile_position=(0, 32 * i),
                                )
                                nc.tensor.matmul(
                                    out_blk, lhsT=wk1, rhs=tgt1[:, j, o : o + MT],
                                    start=False, stop=True, tile_position=(0, 32 * i),
                                )
                            nc.vector.tensor_scalar_add(K4_sb[:, g, :], ps, bqk[:, 1:2])
                        # Q: proj into partition block 0, then replicate via DMA
                        for j in range(NQC):
                            ps = pk.tile([CQK, QC], f32, tag="psq")
                            nc.tensor.matmul(
                                ps, lhsT=wq0, rhs=srcq0[:, j, :], start=True, stop=False
                            )
                            nc.tensor.matmul(
                                ps, lhsT=wq1, rhs=srcq1[:, j, :], start=False, stop=True
                            )
                            nc.vector.tensor_scalar_add(
                                Q4_sb[0:CQK, j, :], ps, bqk[0:CQK, 0:1]
                            )
                        for i in range(1, 4):
                            nc.sync.dma_start(
                                out=Q4_sb[32 * i : 32 * (i + 1), :, :],
                                in_=Q4_sb[0:CQK, :, :],
                            )

                # ---- attention ----
                if STAGE >= 2:
                    with (
                        tc.tile_pool(name="ps_s", bufs=SBUFS, space="PSUM") as ps_s,
                        tc.tile_pool(name="ps_av", bufs=1, space="PSUM") as ps_av,
                        tc.tile_pool(name="ps_l", bufs=1, space="PSUM") as ps_l,
                        tc.tile_pool(name="att", bufs=3) as att,
                        tc.tile_pool(name="outp", bufs=4) as outp,
                    ):
                        for qc in range(NQC):
                            av0 = ps_av.tile([128, QC], f32, tag="av0")
                            av1 = ps_av.tile([128, QC], f32, tag="av1")
                            lrow = ps_l.tile([128, QC], f32, tag="lrow")
                            for g in range(NG):
                                S = ps_s.tile([128, MG, QC], f32, tag="S")
                                for i in range(MG):
                                    mt = g * MG + i
                                    gg, ii = divmod(mt, 4)
                                    blk = slice(32 * ii, 32 * (ii + 1))
                                    nc.tensor.matmul(
                                        S[:, i, :],
                                        lhsT=K4_sb[blk, gg, :],
                                        rhs=Q4_sb[blk, qc, :],
                                        start=True,
                                        stop=True,
                                        tile_position=(32 * ii, 0),
                                    )
                                expT = att.tile([128, MG, QC], at_dt, tag="expT")
                                if STAGE >= 3 and EXPSPLIT:
                                    h = MG // 2
                                    for e in range(2):
                                        nc.scalar.activation(
                                            out=expT[:, e * h : (e + 1) * h, :].rearrange(
                                                "p a b -> p (a b)"
                                            ),
                                            in_=S[:, e * h : (e + 1) * h, :].rearrange(
                                                "p a b -> p (a b)"
                                            ),
                                            func=mybir.ActivationFunctionType.Exp,
                                        )
                                elif STAGE >= 3:
                                    nc.scalar.activation(
                                        out=expT.rearrange("p a b -> p (a b)"),
                                        in_=S.rearrange("p a b -> p (a b)"),
                                        func=mybir.ActivationFunctionType.Exp,
                                    )
                                if STAGE >= 4 and FP8AV:
                                    for t in range(MG // 2):
                                        mt0 = g * MG + 2 * t
                                        first = mt0 == 0
                                        last = mt0 == NMT - 2
                                        for av, cs in ((av0, slice(0, 128)), (av1, slice(128, 256))):
                                            nc.tensor.matmul(
                                                av,
                                                lhsT=VT_sb[:, mt0 : mt0 + 2, cs],
                                                rhs=expT[:, 2 * t : 2 * t + 2, :],
                                                start=first,
                                                stop=last,
                                                perf_mode=mybir.MatmulPerfMode.DoubleRow,
                                            )
                                if STAGE >= 4 and not FP8AV:
                                    for i in range(MG):
                                        mt = g * MG + i
                                        first = mt == 0
                                        last = mt == NMT - 1
                                        nc.tensor.matmul(
                                            av0,
                                            lhsT=VT_sb[:, mt, 0:128],
                                            rhs=expT[:, i, :],
                                            start=first,
                                            stop=last,
                                        )
                                        nc.tensor.matmul(
                                            av1,
                                            lhsT=VT_sb[:, mt, 128:256],
                                            rhs=expT[:, i, :],
                                            start=first,
                                            stop=last,
                                        )
                                if STAGE >= 5:
                                    # denominator partials: adjacent col-packed
                                    # matmuls -> rows 32*i of one PSUM bank
                                    for i in range(MG):
                                        nc.tensor.matmul(
                                            lrow[32 * i : 32 * i + 1, :],
                                            lhsT=ones_col,
                                            rhs=expT[:, i, :],
                                            start=g == 0,
                                            stop=g == NG - 1,
                                            tile_position=(0, 32 * i),
                                        )
                            if STAGE < 6:
                                continue
                            # free the av/lrow banks fast: copy to SBUF, then
                            # run the fold/recip/normalize tail asynchronously
                            av0_sb = outp.tile([128, QC], f32, tag="av0_sb")
                            av1_sb = outp.tile([128, QC], f32, tag="av1_sb")
                            nc.vector.tensor_copy(out=av0_sb, in_=av0)
                            nc.vector.tensor_copy(out=av1_sb, in_=av1)
                            for i in range(MG):
                                nc.vector.tensor_copy(
                                    out=lrow_sb[32 * i : 32 * i + 1, :],
                                    in_=lrow[32 * i : 32 * i + 1, :],
                                )
                            l_rep = outp.tile([128, QC], f32, tag="l_rep")
                            r_rep = outp.tile([128, QC], f32, tag="r_rep")
                            nc.gpsimd.partition_all_reduce(
                                l_rep, lrow_sb, 128, bass_isa.ReduceOp.add
                            )
                            nc.vector.reciprocal_approx_fast(out=r_rep, in_=l_rep)
                            for ci, (av_sb, srcr) in enumerate(
                                ((av0_sb, srcr0), (av1_sb, srcr1))
                            ):
                                o = outp.tile([128, QC], f32, tag=f"o{ci}")
                                nc.vector.tensor_mul(o, av_sb, r_rep)
                                nc.vector.tensor_add(o, o, srcr[:, qc, :])
                                nc.sync.dma_start(
                                    out=out_d[
                                        128 * ci : 128 * (ci + 1),
                                        qc * QC : (qc + 1) * QC,
                                    ],
                                    in_=o,
                                )
    nc.compile()
    return nc


_cached = None


def _get_bass():
    global _cached
    if _cached is None:
        _cached = _build_bass()
    return _cached


def kernel(src_feat, tgt_feat, Wq, bq, Wk, bk, Wv, bv):
    """Full inputs in, full output out. Shards internally across 8 cores."""
    global _last_results
    from concourse.bass_utils import run_bass_kernel_spmd

    src = np.asarray(src_feat, dtype=np.float32).reshape(B, C, N)
    tgt = np.asarray(tgt_feat, dtype=np.float32).reshape(B, C, N)
    wqT = np.ascontiguousarray(np.asarray(Wq, np.float32).T).astype(BF16)
    wkT = np.ascontiguousarray(np.asarray(Wk, np.float32).T).astype(BF16)
    wvT = np.ascontiguousarray(np.asarray(Wv, np.float32).T).astype(BF16)
    bqk = np.tile(
        np.stack([np.asarray(bq, np.float32), np.asarray(bk, np.float32)], axis=1),
        (4, 1),
    )  # [128, 2]
    bvr = np.asarray(bv, np.float32).reshape(1, C).astype(BF16)

    tgt_bf = tgt.astype(BF16)

    in_maps = []
    for c in range(NCORES):
        b, h = divmod(c, 2)
        qsl = slice(h * QSH, (h + 1) * QSH)
        in_maps.append(
            {
                "tgt": np.ascontiguousarray(tgt_bf[b]),
                "srcq": np.ascontiguousarray(src[b, :, qsl]).astype(BF16),
                "srcr": np.ascontiguousarray(src[b, :, qsl]),
                "wqT": wqT,
                "wkT": wkT,
                "wvT": wvT,
                "bqk": np.ascontiguousarray(bqk),
                "bv": bvr,
            }
        )

    nc = _get_bass()
    res = None
    for attempt in range(3):
        try:
            res = run_bass_kernel_spmd(
                nc,
                in_maps,
                core_ids=list(range(NCORES)),
                trace=bool(int(os.environ.get("KERNEL_TRACE", "0"))),
            )
            break
        except Exception:
            # the axon-tunneled devices occasionally report
            # NRT_EXEC_UNIT_UNRECOVERABLE; a retry on a fresh execute recovers
            if attempt == 2:
                raise
            import time as _time

            _time.sleep(5)
    _last_results = res

    out = np.empty((B, C, N), dtype=np.float32)
    for c in range(NCORES):
        b, h = divmod(c, 2)
        out[b, :, h * QSH : (h + 1) * QSH] = res.results[c]["out"]
    return out.reshape(B, C, H, W)


# revision 36
# speedup vs baseline: 21304.3179x; 1.0955x over previous
"""CrossViewTransformer kernel for 8 Trainium2 NeuronCores.

Problem: B=4, C=256, H=W=64 (N=4096), Cqk=32 cross-attention + residual.
  Q = Wq@src, K = Wk@tgt, V = Wv@tgt  (1x1 convs over channels)
  out = softmax(Q^T K) @ V^T + src     (no 1/sqrt(d) scale)

Sharding: 8 cores = 4 batches x 2 query-halves. Each core computes attention
for 2048 queries x 4096 keys of one batch. The V projection is replicated
across the 2 cores of a batch (cheap: 0.5 GFLOP) while the expensive parts
(scores, exp, attn@V) are fully split.

Per-core pipeline (per 512-query chunk, m = key index, groups of MG=2
m-tiles):
  scoresT[m,q]: row-packed matmuls (K=32 contraction at row groups 32*i)
    into a double-buffered PSUM tile; one Exp activation per group writes
    fp8e4m3 attention weights to SBUF (no max-subtraction needed: scores
    are O(1) by construction, exp never overflows)
  attn@V: fp8 DoubleRow matmuls (pairs of m-tiles per instruction)
    accumulating [c_half, q] in PSUM across all 32 m-tiles
  denominator: one DoubleRow ones-matmul per group (both m-tiles at half
    rate) accumulating a [1, QC] PSUM row; folded+broadcast by one gpsimd
    partition_all_reduce, then a custom-DVE approximate reciprocal gives
    1/l replicated on all partitions
  out = av * r + src_res -> DRAM  (residual dominates the output, which is
    why fp8 attention weights cost only ~2e-4 relative error)
"""

import os
import sys

sys.path.insert(0, "/opt/trn_rl_repo")

import numpy as np
import ml_dtypes

BF16 = ml_dtypes.bfloat16

B, C, H, W = 4, 256, 64, 64
N = H * W            # 4096 keys (and queries per batch)
CQK = 32
NCORES = 8
QSH = N // 2         # 2048 queries per core
QC = 512             # q-chunk width (one PSUM bank)
NQC = QSH // QC      # 4 q-chunks
MT = 128             # m-tile (keys per scoresT tile)
NMT = N // MT        # 32 m-tiles
MG = int(os.environ.get("KERNEL_MG", "2"))   # m-tiles per exp group (<=4)
NG = NMT // MG       # groups per q-chunk
SBUFS = int(os.environ.get("KERNEL_SBUFS", "2"))
LOOP = int(os.environ.get("KERNEL_LOOP", "0"))  # >0: repeat body for timing
# timing bisection: 0=empty loop body, 1=+proj, 2=+QK, 3=+exp, 4=+AV,
# 5=+L matmuls, 6=full
STAGE = int(os.environ.get("KERNEL_STAGE", "6"))
# split exp into 2 half-group ops so QK(g+1) overlaps exp(g) in one S tile
EXPSPLIT = bool(int(os.environ.get("KERNEL_EXPSPLIT", "0")))
# fp8e4m3 attn weights + V with DoubleRow attn@V matmuls (2x PE rate)
FP8AV = bool(int(os.environ.get("KERNEL_FP8AV", "1")))

_last_results = None  # BassKernelResults of the most recent run (for test.py)


def _build_bass():
    import concourse.bass as bass
    import concourse.tile as tile
    from concourse import bacc, mybir
    from concourse import bass_isa
    from contextlib import ExitStack

    f32 = mybir.dt.float32
    bf16 = mybir.dt.bfloat16
    fp8 = mybir.dt.float8e4
    at_dt = fp8 if FP8AV else bf16  # attn-weight / V dtype
    AT_NP = ml_dtypes.float8_e4m3 if FP8AV else BF16

    nc = bacc.Bacc("TRN2")

    # ---- DRAM I/O ----
    tgt_d = nc.dram_tensor("tgt", [C, N], bf16, kind="ExternalInput")
    srcq_d = nc.dram_tensor("srcq", [C, QSH], bf16, kind="ExternalInput")
    srcr_d = nc.dram_tensor("srcr", [C, QSH], f32, kind="ExternalInput")
    wqT_d = nc.dram_tensor("wqT", [C, CQK], bf16, kind="ExternalInput")
    wkT_d = nc.dram_tensor("wkT", [C, CQK], bf16, kind="ExternalInput")
    wvT_d = nc.dram_tensor("wvT", [C, C], bf16, kind="ExternalInput")
    bqk_d = nc.dram_tensor("bqk", [128, 2], f32, kind="ExternalInput")
    bv_d = nc.dram_tensor("bv", [1, C], bf16, kind="ExternalInput")
    out_d = nc.dram_tensor("out", [C, QSH], f32, kind="ExternalOutput")

    ones_col_d = nc.inline_tensor(np.ones((128, 1), dtype=AT_NP), name="ones_col")
    ones2_d = nc.inline_tensor(np.ones((128, 32), dtype=AT_NP), name="ones2")

    with tile.TileContext(nc) as tc:
        with (
            tc.tile_pool(name="const", bufs=1) as const,
            tc.tile_pool(name="data", bufs=1) as data,
        ):
            # ---- ACT table warmup: a dependency-free Exp so walrus's
            # inserted ACT_TABLE_LOAD lands on an instruction with <=1 wait.
            warm = const.tile([1, 8], f32, tag="warm")
            nc.vector.memset(warm, 0.0)
            nc.scalar.activation(
                out=warm, in_=warm, func=mybir.ActivationFunctionType.Exp
            )

            # ---- constants / weights ----
            wq0 = const.tile([128, CQK], bf16, tag="wq0")
            wq1 = const.tile([128, CQK], bf16, tag="wq1")
            wk0 = const.tile([128, CQK], bf16, tag="wk0")
            wk1 = const.tile([128, CQK], bf16, tag="wk1")
            wv0 = const.tile([128, C], bf16, tag="wv0")
            wv1 = const.tile([128, C], bf16, tag="wv1")
            bqk = const.tile([128, 2], f32, tag="bqk")
            bvr = const.tile([1, C], bf16, tag="bvr")
            ones_col = const.tile([128, 1], at_dt, tag="ones_col")
            ones2 = const.tile([128, 32], at_dt, tag="ones2")
            nc.sync.dma_start(out=wq0, in_=wqT_d[0:128, :])
            nc.sync.dma_start(out=wq1, in_=wqT_d[128:256, :])
            nc.sync.dma_start(out=wk0, in_=wkT_d[0:128, :])
            nc.sync.dma_start(out=wk1, in_=wkT_d[128:256, :])
            nc.sync.dma_start(out=wv0, in_=wvT_d[0:128, :])
            nc.sync.dma_start(out=wv1, in_=wvT_d[128:256, :])
            nc.sync.dma_start(out=bqk, in_=bqk_d[:, :])
            nc.sync.dma_start(out=bvr, in_=bv_d[:, :])
            nc.sync.dma_start(out=ones_col, in_=ones_col_d[:, :])
            nc.sync.dma_start(out=ones2, in_=ones2_d[:, :])

            # ---- big data tiles ----
            tgt0 = data.tile([128, 8, 512], bf16, tag="tgt0")
            tgt1 = data.tile([128, 8, 512], bf16, tag="tgt1")
            for j in range(8):
                sl = slice(j * 512, (j + 1) * 512)
                nc.sync.dma_start(out=tgt0[:, j, :], in_=tgt_d[0:128, sl])
                nc.sync.dma_start(out=tgt1[:, j, :], in_=tgt_d[128:256, sl])
            srcq0 = data.tile([128, NQC, QC], bf16, tag="srcq0")
            srcq1 = data.tile([128, NQC, QC], bf16, tag="srcq1")
            srcr0 = data.tile([128, NQC, QC], f32, tag="srcr0")
            srcr1 = data.tile([128, NQC, QC], f32, tag="srcr1")
            for j in range(NQC):
                sl = slice(j * QC, (j + 1) * QC)
                nc.sync.dma_start(out=srcq0[:, j, :], in_=srcq_d[0:128, sl])
                nc.sync.dma_start(out=srcq1[:, j, :], in_=srcq_d[128:256, sl])
                nc.sync.dma_start(out=srcr0[:, j, :], in_=srcr_d[0:128, sl])
                nc.sync.dma_start(out=srcr1[:, j, :], in_=srcr_d[128:256, sl])

            # persistent staging tile for the denominator fold: rows 0/32
            # carry the two col-group partials, the rest stay zero
            lrow_sb = data.tile([128, QC], f32, tag="lrow_sb")
            nc.vector.memset(lrow_sb, 0.0)

            # bv broadcast to all partitions once (for the VT copy+bias add)
            bv_rep = data.tile([128, C], bf16, tag="bv_rep")
            nc.gpsimd.partition_broadcast(bv_rep, bvr)

            # K4_sb[32*i + cqk, g, col] = K[cqk, (4g+i)*128 + col]
            # (4 m-tiles of a group live at partition blocks 0/32/64/96)
            K4_sb = data.tile([128, N // (4 * MT), MT], bf16, tag="K4_sb")
            # Q replicated at all 4 partition blocks
            Q4_sb = data.tile([128, NQC, QC], bf16, tag="Q4_sb")
            VT_sb = data.tile([128, NMT, C], at_dt, tag="VT_sb")

            body_stack = ExitStack()
            if LOOP:
                body_stack.enter_context(tc.For_i(0, LOOP, 1))
            with body_stack:
                if STAGE == 0:
                    tick = data.tile([1, 8], f32, tag="tick")
                    nc.vector.memset(tick, 1.0)
                # ---- projections ----
                if STAGE >= 1:
                    with (
                        tc.tile_pool(name="pv", bufs=3, space="PSUM") as pv,
                        tc.tile_pool(name="pk", bufs=2, space="PSUM") as pk,
                    ):
                        # V^T tiles: VT[m,c] = sum_ch tgt[ch,m] WvT[ch,c] (+ bv)
                        for mt in range(NMT):
                            ps = pv.tile([128, C], f32, tag="psv")
                            j, o = divmod(mt * MT, 512)
                            lhs0 = tgt0[:, j, o : o + MT]
                            lhs1 = tgt1[:, j, o : o + MT]
                            nc.tensor.matmul(
                                ps, lhsT=lhs0, rhs=wv0, start=True, stop=False
                            )
                            nc.tensor.matmul(
                                ps, lhsT=lhs1, rhs=wv1, start=False, stop=True
                            )
                            nc.vector.tensor_add(VT_sb[:, mt, :], ps, bv_rep)
                        # K: 4 m-tiles per group at partition blocks (col groups)
                        for g in range(N // (4 * MT)):
                            ps = pk.tile([128, MT], f32, tag="psk")
                            for i in range(4):
                                mt = 4 * g + i
                                j, o = divmod(mt * MT, 512)
                                out_blk = ps[32 * i : 32 * (i + 1), :]
                                nc.tensor.matmul(
                                    out_blk, lhsT=wk0, rhs=tgt0[:, j, o : o + MT],
                                    start=True, stop=False, tile_position=(0, 32 * i),
                                )
                                nc.tensor.matmul(
                                    out_blk, lhsT=wk1, rhs=tgt1[:, j, o : o + MT],
                                    start=False, stop=True, tile_position=(0, 32 * i),
                                )
                            nc.vector.tensor_scalar_add(K4_sb[:, g, :], ps, bqk[:, 1:2])
                        # Q: proj into partition block 0, then replicate via DMA
                        for j in range(NQC):
                            ps = pk.tile([CQK, QC], f32, tag="psq")
                            nc.tensor.matmul(
                                ps, lhsT=wq0, rhs=srcq0[:, j, :], start=True, stop=False
                            )
                            nc.tensor.matmul(
                                ps, lhsT=wq1, rhs=srcq1[:, j, :], start=False, stop=True
                            )
                            nc.vector.tensor_scalar_add(
                                Q4_sb[0:CQK, j, :], ps, bqk[0:CQK, 0:1]
                            )
                        for i in range(1, 4):
                            nc.sync.dma_start(
                                out=Q4_sb[32 * i : 32 * (i + 1), :, :],
                                in_=Q4_sb[0:CQK, :, :],
                            )

                # ---- attention ----
                if STAGE >= 2:
                    with (
                        tc.tile_pool(name="ps_s", bufs=SBUFS, space="PSUM") as ps_s,
                        tc.tile_pool(name="ps_av", bufs=1, space="PSUM") as ps_av,
                        tc.tile_pool(name="ps_l", bufs=1, space="PSUM") as ps_l,
                        tc.tile_pool(name="att", bufs=4) as att,
                        tc.tile_pool(name="outp", bufs=4) as outp,
                    ):
                        def emit_qk(qc, g):
                            S = ps_s.tile([128, MG, QC], f32, tag="S")
                            for i in range(MG):
                                mt = g * MG + i
                                gg, ii = divmod(mt, 4)
                                blk = slice(32 * ii, 32 * (ii + 1))
                                nc.tensor.matmul(
                                    S[:, i, :],
                                    lhsT=K4_sb[blk, gg, :],
                                    rhs=Q4_sb[blk, qc, :],
                                    start=True,
                                    stop=True,
                                    tile_position=(32 * ii, 0),
                                )
                            return S

                        groups = [(qc, g) for qc in range(NQC) for g in range(NG)]
                        av0 = av1 = lrow = None
                        for idx, (qc, g) in enumerate(groups):
                            if g == 0:
                                av0 = ps_av.tile([128, QC], f32, tag="av0")
                                av1 = ps_av.tile([128, QC], f32, tag="av1")
                                lrow = ps_l.tile([128, QC], f32, tag="lrow")
                            S_cur = emit_qk(qc, g)
                            expT = att.tile([128, MG, QC], at_dt, tag="expT")
                            if STAGE >= 3:
                                nc.scalar.activation(
                                    out=expT.rearrange("p a b -> p (a b)"),
                                    in_=S_cur.rearrange("p a b -> p (a b)"),
                                    func=mybir.ActivationFunctionType.Exp,
                                )
                            if STAGE >= 4 and FP8AV:
                                for t in range(MG // 2):
                                    mt0 = g * MG + 2 * t
                                    first = mt0 == 0
                                    last = mt0 == NMT - 2
                                    for av, cs in (
                                        (av0, slice(0, 128)),
                                        (av1, slice(128, 256)),
                                    ):
                                        nc.tensor.matmul(
                                            av,
                                            lhsT=VT_sb[:, mt0 : mt0 + 2, cs],
                                            rhs=expT[:, 2 * t : 2 * t + 2, :],
                                            start=first,
                                            stop=last,
                                            perf_mode=mybir.MatmulPerfMode.DoubleRow,
                                        )
                            if STAGE >= 4 and not FP8AV:
                                for i in range(MG):
                                    mt = g * MG + i
                                    first = mt == 0
                                    last = mt == NMT - 1
                                    nc.tensor.matmul(
                                        av0,
                                        lhsT=VT_sb[:, mt, 0:128],
                                        rhs=expT[:, i, :],
                                        start=first,
                                        stop=last,
                                    )
                                    nc.tensor.matmul(
                                        av1,
                                        lhsT=VT_sb[:, mt, 128:256],
                                        rhs=expT[:, i, :],
                                        start=first,
                                        stop=last,
                                    )
                            if STAGE >= 5 and FP8AV and MG == 2:
                                # denominator: one DoubleRow matmul contracts
                                # both m-tiles of the group at half rate
                                nc.tensor.matmul(
                                    lrow[0:1, :],
                                    lhsT=ones2.rearrange("p (a b) -> p a b", b=16)[:, :, 0:1],
                                    rhs=expT[:, 0:2, :],
                                    start=g == 0,
                                    stop=g == NG - 1,
                                    perf_mode=mybir.MatmulPerfMode.DoubleRow,
                                )
                            elif STAGE >= 5:
                                for i in range(MG):
                                    nc.tensor.matmul(
                                        lrow[32 * i : 32 * i + 1, :],
                                        lhsT=ones_col,
                                        rhs=expT[:, i, :],
                                        start=g == 0,
                                        stop=g == NG - 1,
                                        tile_position=(0, 32 * i),
                                    )
                            if STAGE < 6 or g != NG - 1:
                                continue
                            # free the av/lrow banks fast: copy to SBUF, then
                            # run the fold/recip/normalize tail asynchronously
                            av0_sb = outp.tile([128, QC], f32, tag="av0_sb")
                            av1_sb = outp.tile([128, QC], f32, tag="av1_sb")
                            nc.vector.tensor_copy(out=av0_sb, in_=av0)
                            nc.vector.tensor_copy(out=av1_sb, in_=av1)
                            nrows = 1 if (FP8AV and MG == 2) else MG
                            for i in range(nrows):
                                nc.vector.tensor_copy(
                                    out=lrow_sb[32 * i : 32 * i + 1, :],
                                    in_=lrow[32 * i : 32 * i + 1, :],
                                )
                            l_rep = outp.tile([128, QC], f32, tag="l_rep")
                            r_rep = outp.tile([128, QC], f32, tag="r_rep")
                            nc.gpsimd.partition_all_reduce(
                                l_rep, lrow_sb, 128, bass_isa.ReduceOp.add
                            )
                            nc.vector.reciprocal_approx_fast(out=r_rep, in_=l_rep)
                            for ci, (av_sb, srcr) in enumerate(
                                ((av0_sb, srcr0), (av1_sb, srcr1))
                            ):
                                o = outp.tile([128, QC], f32, tag=f"o{ci}")
                                nc.vector.tensor_mul(o, av_sb, r_rep)
                                nc.vector.tensor_add(o, o, srcr[:, qc, :])
                                nc.sync.dma_start(
                                    out=out_d[
                                        128 * ci : 128 * (ci + 1),
                                        qc * QC : (qc + 1) * QC,
                                    ],
                                    in_=o,
                                )
    nc.compile()
    return nc


_cached = None


def _get_bass():
    global _cached
    if _cached is None:
        _cached = _build_bass()
    return _cached


def kernel(src_feat, tgt_feat, Wq, bq, Wk, bk, Wv, bv):
    """Full inputs in, full output out. Shards internally across 8 cores."""
    global _last_results
    from concourse.bass_utils import run_bass_kernel_spmd

    src = np.asarray(src_feat, dtype=np.float32).reshape(B, C, N)
    tgt = np.asarray(tgt_feat, dtype=np.float32).reshape(B, C, N)
    wqT = np.ascontiguousarray(np.asarray(Wq, np.float32).T).astype(BF16)
    wkT = np.ascontiguousarray(np.asarray(Wk, np.float32).T).astype(BF16)
    wvT = np.ascontiguousarray(np.asarray(Wv, np.float32).T).astype(BF16)
    bqk = np.tile(
        np.stack([np.asarray(bq, np.float32), np.asarray(bk, np.float32)], axis=1),
        (4, 1),
    )  # [128, 2]
    bvr = np.asarray(bv, np.float32).reshape(1, C).astype(BF16)

    tgt_bf = tgt.astype(BF16)

    in_maps = []
    for c in range(NCORES):
        b, h = divmod(c, 2)
        qsl = slice(h * QSH, (h + 1) * QSH)
        in_maps.append(
            {
                "tgt": np.ascontiguousarray(tgt_bf[b]),
                "srcq": np.ascontiguousarray(src[b, :, qsl]).astype(BF16),
                "srcr": np.ascontiguousarray(src[b, :, qsl]),
                "wqT": wqT,
                "wkT": wkT,
                "wvT": wvT,
                "bqk": np.ascontiguousarray(bqk),
                "bv": bvr,
            }
        )

    nc = _get_bass()
    res = None
    for attempt in range(3):
        try:
            res = run_bass_kernel_spmd(
                nc,
                in_maps,
                core_ids=list(range(NCORES)),
                trace=bool(int(os.environ.get("KERNEL_TRACE", "0"))),
            )
            break
        except Exception:
            # the axon-tunneled devices occasionally report
            # NRT_EXEC_UNIT_UNRECOVERABLE; a retry on a fresh execute recovers
            if attempt == 2:
                raise
            import time as _time

            _time.sleep(5)
    _last_results = res

    out = np.empty((B, C, N), dtype=np.float32)
    for c in range(NCORES):
        b, h = divmod(c, 2)
        out[b, :, h * QSH : (h + 1) * QSH] = res.results[c]["out"]
    return out.reshape(B, C, H, W)
